# revision 1
# baseline (speedup 1.0000x reference)
"""Batched 3-layer GCN (nn_BatchGCN) on 8 TRN2 NeuronCores — one graph per core.

Math per graph, per layer:  h' = Ahat @ (h @ W.T + b),  Ahat = D^-1/2 A D^-1/2
(duplicate edges accumulate; relu between layers 1-2).  The symmetric
normalization factors node-wise:
    out[i] = dinv[i] * sum_{e: row_e==i} (dinv * z)[col_e],  z = h@W.T + b
so no per-edge scalar work is needed — only dense node-wise scaling.

Device algorithm (per core; no per-edge DMA scatter — scatter is done with
TensorEngine one-hot matmuls, which is deterministic and avoids the
duplicate-index hazard of DMA scatter-add):
  - host buckets the edge list by destination block (128 nodes), CB chunks of
    128 edge slots per block; dummy slots carry row-offset 999 so their
    one-hot column is all-zero and they contribute nothing
  - degree pass: per block, one-hot [128e x 128d] matmuls against an all-ones
    rhs accumulate deg (replicated across the 64 feature cols) in PSUM;
    dinv = (deg>0)/sqrt(max(deg,1))
  - per layer: z^T = W_T.T @ h^T on PE (feat-major, packed two node-halves on
    128 partitions), +bias on DVE, PE-transpose to node-major, dinv scaling
    fused into the ScalarEngine copy (scale=), one contiguous DMA of z~ to a
    partition-major DRAM layout; dma_gather fetches the 256B messages per
    4-block chunk (the only indexed DMA; gathers are race-free); DVE builds
    the one-hot for each block with a single broadcast-AP is_equal; CB
    matmuls accumulate the block's segment sum in PSUM; relu*dinv is fused
    into the ScalarEngine activation that drains PSUM.
All compute is f32; results match the f32 reference to ~1e-5 relative.

Host-side work is index/layout marshaling only (transpose/pad/bucket the
given arrays); all arithmetic on tensor values happens on-device.
"""
from dataclasses import dataclass

import numpy as np

import concourse.bacc as bacc
import concourse.mybir as mybir
import concourse.tile as tile
from concourse.bass import broadcast_tensor_aps
from concourse.bass_utils import run_bass_kernel_spmd
from concourse.library_config import mlp

B, NV, E, F = 8, 10000, 160000, 64
N = 10240          # padded node count (80 blocks of 128)
CORES = list(range(8))


@dataclass(frozen=True)
class _Cfg:
    N: int
    NV: int
    F: int
    CB: int        # chunks (of 128 edge slots) per destination block
    GB: int        # blocks per gather chunk
    layers: int = 3

    @property
    def nblk(self):
        return self.N // 128

    @property
    def epad(self):
        return self.nblk * self.CB * 128

    @property
    def ngch(self):
        return self.nblk // self.GB


def _build(cfg: _Cfg):
    N, F, CB = cfg.N, cfg.F, cfg.CB
    NBLK, EPAD, GB, NGCH = cfg.nblk, cfg.epad, cfg.GB, cfg.ngch
    NQ = N // 512

    nc = bacc.Bacc("TRN2", debug=False)
    x_hbm = nc.dram_tensor("x_packed", [128, N // 2], mybir.dt.float32, kind="ExternalInput")
    w_hbm = nc.dram_tensor("w_t", [128, cfg.layers * F], mybir.dt.float32, kind="ExternalInput")
    b_hbm = nc.dram_tensor("bias", [128, cfg.layers], mybir.dt.float32, kind="ExternalInput")
    i_hbm = nc.dram_tensor("ident", [128, 128], mybir.dt.float32, kind="ExternalInput")
    t_hbm = nc.dram_tensor("iota_t", [128, CB * 128], mybir.dt.float32, kind="ExternalInput")
    r_hbm = nc.dram_tensor("rowoff", [128, EPAD // 128], mybir.dt.float32, kind="ExternalInput")
    c_hbm = nc.dram_tensor("colr", [128, EPAD // 16], mybir.dt.int16, kind="ExternalInput")
    out_hbm = nc.dram_tensor("out_pm", [128, NBLK * F], mybir.dt.float32, kind="ExternalOutput")
    zdram = nc.dram_tensor("zdram", [N, F], mybir.dt.float32)

    with tile.TileContext(nc) as tc:
        with (
            tc.tile_pool(name="const", bufs=1) as cp,
            tc.tile_pool(name="state", bufs=1) as sp,
            tc.tile_pool(name="oh", bufs=2) as ohp,
            tc.tile_pool(name="msg", bufs=2) as mp,
            tc.tile_pool(name="zb", bufs=2) as zp,
            tc.tile_pool(name="pz", bufs=2, space="PSUM") as pz,
            tc.tile_pool(name="pt", bufs=2, space="PSUM") as pt,
            tc.tile_pool(name="psc", bufs=2, space="PSUM") as psc,
            tc.tile_pool(name="pbt", bufs=2, space="PSUM") as pbt,
        ):
            nc.gpsimd.load_library(mlp)

            wt = cp.tile([128, cfg.layers, F], mybir.dt.float32)
            nc.sync.dma_start(wt[:], w_hbm[:].rearrange("p (l f) -> p l f", l=cfg.layers))
            bs = cp.tile([128, cfg.layers], mybir.dt.float32)
            nc.sync.dma_start(bs[:], b_hbm[:])
            ident = cp.tile([128, 128], mybir.dt.float32)
            nc.sync.dma_start(ident[:], i_hbm[:])
            iota = cp.tile([128, CB * 128], mybir.dt.float32)
            nc.sync.dma_start(iota[:], t_hbm[:])
            rowoff = cp.tile([128, EPAD // 128], mybir.dt.float32)
            nc.sync.dma_start(rowoff[:], r_hbm[:])
            colr = cp.tile([128, EPAD // 16], mybir.dt.int16)
            nc.sync.dma_start(colr[:], c_hbm[:])
            ones = cp.tile([128, F], mybir.dt.float32)
            nc.vector.memset(ones[:], 1.0)

            hA = sp.tile([128, N // 2], mybir.dt.float32, tag="hA")
            nc.sync.dma_start(hA[:], x_hbm[:])
            hB = sp.tile([128, N // 2], mybir.dt.float32, tag="hB")
            dinv = sp.tile([128, NBLK, F], mybir.dt.float32, tag="dinv")
            stage = sp.tile([128, NBLK, F], mybir.dt.float32, tag="stage")
            # hB doubles as the transient deg>0 mask (consumed before any h write)
            mask = hB[:].rearrange("p (c f) -> p c f", c=NBLK)

            def onehot_for(b):
                # oh[e, c*128+d] = (rowoff[e, b*CB+c] == d)
                oh = ohp.tile([128, CB * 128], mybir.dt.float32, tag="oh")
                ro3 = rowoff[:, b * CB:(b + 1) * CB].rearrange("p c -> p c ()")
                io3 = iota[:].rearrange("p (c d) -> p c d", c=CB)
                a, bb = broadcast_tensor_aps(io3, ro3)
                nc.vector.tensor_tensor(
                    out=oh[:].rearrange("p (c d) -> p c d", c=CB),
                    in0=a, in1=bb, op=mybir.AluOpType.is_equal)
                return oh

            def seg_matmuls(ps, oh, rhs_fn):
                for c in range(CB):
                    nc.tensor.matmul(
                        ps[:], oh[:, c * 128:(c + 1) * 128], rhs_fn(c),
                        start=(c == 0), stop=(c == CB - 1))

            # ---- degree pass ----
            for b in range(NBLK):
                ps = psc.tile([128, F], mybir.dt.float32, tag="psc")
                oh = onehot_for(b)
                seg_matmuls(ps, oh, lambda c: ones[:])
                nc.scalar.copy(stage[:, b], ps[:])

            nc.vector.tensor_scalar(out=mask, in0=stage[:], scalar1=0.5,
                                    scalar2=None, op0=mybir.AluOpType.is_gt)
            nc.vector.tensor_scalar(out=stage[:], in0=stage[:], scalar1=1.0,
                                    scalar2=None, op0=mybir.AluOpType.max)
            nc.scalar.activation(stage[:], stage[:], mybir.ActivationFunctionType.Sqrt)
            nc.vector.reciprocal(stage[:], stage[:])
            nc.vector.tensor_tensor(out=dinv[:], in0=stage[:], in1=mask,
                                    op=mybir.AluOpType.mult)

            hcur = hA
            for lay in range(cfg.layers):
                hnxt = hB if hcur is hA else hA
                for q in range(NQ):
                    half = 0 if q < NQ // 2 else 64
                    qq = q % (NQ // 2)
                    pzt = pz.tile([64, 512], mybir.dt.float32, tag="pz")
                    nc.tensor.matmul(
                        pzt[:], wt[half:half + 64, lay],
                        hcur[half:half + 64, qq * 512:(qq + 1) * 512],
                        start=True, stop=True)
                    zb = zp.tile([64, 512], mybir.dt.float32, tag="zb")
                    nc.vector.tensor_scalar(
                        out=zb[:], in0=pzt[:],
                        scalar1=bs[half:half + 64, lay:lay + 1],
                        scalar2=None, op0=mybir.AluOpType.add)
                    for j in range(4):
                        blk = 4 * q + j
                        ptt = pt.tile([128, F], mybir.dt.float32, tag="pt")
                        nc.tensor.transpose(
                            ptt[:], zb[:, j * 128:(j + 1) * 128], ident[:64, :64])
                        nc.scalar.activation(
                            stage[:, blk], ptt[:],
                            mybir.ActivationFunctionType.Copy,
                            scale=dinv[:, blk, 0:1])
                nc.sync.dma_start(
                    zdram[:].rearrange("(p c) f -> p c f", p=128), stage[:])
                for g in range(NGCH):
                    nidx = GB * CB * 128
                    msgs = mp.tile([128, GB * CB, F], mybir.dt.float32, tag="msgs")
                    nc.gpsimd.dma_gather(
                        msgs[:], zdram[:],
                        colr[:, g * (nidx // 16):(g + 1) * (nidx // 16)],
                        nidx, nidx, F, single_packet=False)
                    for bb in range(GB):
                        b = g * GB + bb
                        ps = psc.tile([128, F], mybir.dt.float32, tag="psc")
                        oh = onehot_for(b)
                        seg_matmuls(ps, oh, lambda c, _bb=bb: msgs[:, _bb * CB + c])
                        if lay < cfg.layers - 1:
                            hm = zp.tile([128, F], mybir.dt.float32, tag="hm")
                            nc.scalar.activation(
                                hm[:], ps[:], mybir.ActivationFunctionType.Relu,
                                scale=dinv[:, b, 0:1])
                            pbtt = pbt.tile([64, 128], mybir.dt.float32, tag="pbt")
                            nc.tensor.transpose(pbtt[:], hm[:], ident[:])
                            half = 0 if b < NBLK // 2 else 64
                            bq = b % (NBLK // 2)
                            nc.scalar.copy(
                                hnxt[half:half + 64, bq * 128:(bq + 1) * 128], pbtt[:])
                        else:
                            nc.scalar.activation(
                                stage[:, b], ps[:],
                                mybir.ActivationFunctionType.Copy,
                                scale=dinv[:, b, 0:1])
                hcur = hnxt
            nc.sync.dma_start(
                out_hbm[:].rearrange("p (c f) -> p c f", c=NBLK), stage[:])

    nc.compile()
    return nc


def _prep_inputs(cfg: _Cfg, x, edge_index, Ws, bs_):
    """Index/layout marshaling for one graph (no value arithmetic)."""
    N, F, CB, NV = cfg.N, cfg.F, cfg.CB, cfg.NV
    NBLK, EPAD = cfg.nblk, cfg.epad
    row = np.asarray(edge_index[:, 0], np.int64)
    col = np.asarray(edge_index[:, 1], np.int64)
    blk = row >> 7
    order = np.argsort(blk, kind="stable")
    counts = np.bincount(blk, minlength=NBLK)
    assert counts.max() <= CB * 128, f"block overflow: {counts.max()}"
    starts = np.cumsum(counts) - counts
    base = np.repeat(np.arange(NBLK) * CB * 128, counts)
    within = np.arange(len(row)) - np.repeat(starts, counts)
    slots = base + within
    rowoff = np.full(EPAD, 999.0, np.float32)
    colv = np.zeros(EPAD, np.int64)
    rowoff[slots] = (row & 127)[order]
    colv[slots] = col[order]
    # remap node id -> row of the partition-major z~ DRAM layout
    colr = ((colv & 127) * NBLK + (colv >> 7)).astype(np.int16)

    def wrap16(a):
        w = a.reshape(-1, 16).T
        return np.tile(w, (8, 1))

    rowoff_t = np.ascontiguousarray(rowoff.reshape(-1, 128).T)
    colr_t = wrap16(colr)

    xT = np.zeros((64, N), np.float32)
    xT[:, :NV] = np.asarray(x, np.float32).T
    x_packed = np.concatenate([xT[:, :N // 2], xT[:, N // 2:]], axis=0)

    w_t = np.zeros((128, len(Ws), F), np.float32)
    bias = np.zeros((128, len(Ws)), np.float32)
    for l, (W, b) in enumerate(zip(Ws, bs_)):
        w_t[:64, l] = np.asarray(W, np.float32).T
        w_t[64:, l] = np.asarray(W, np.float32).T
        bias[:64, l] = np.asarray(b, np.float32)
        bias[64:, l] = np.asarray(b, np.float32)

    return {
        "x_packed": x_packed,
        "w_t": np.ascontiguousarray(w_t.reshape(128, -1)),
        "bias": bias,
        "ident": np.eye(128, dtype=np.float32),
        "iota_t": np.tile(np.tile(np.arange(128, dtype=np.float32), CB), (128, 1)),
        "rowoff": rowoff_t,
        "colr": colr_t,
    }


def _unpack_output(cfg: _Cfg, out_pm):
    o = out_pm.reshape(128, cfg.nblk, cfg.F).transpose(1, 0, 2).reshape(cfg.N, cfg.F)
    return o[:cfg.NV]


def kernel(x, edge_index, W1, b1, W2, b2, W3, b3):
    x = np.asarray(x)
    edge_index = np.asarray(edge_index)
    Ws = [np.asarray(W1), np.asarray(W2), np.asarray(W3)]
    bs_ = [np.asarray(b1), np.asarray(b2), np.asarray(b3)]
    nb = x.shape[0]
    assert x.shape == (B, NV, F) and edge_index.shape == (B, E, 2)

    # destination-block budget (recompiles only for pathological inputs)
    maxcnt = max(
        int(np.bincount(np.asarray(edge_index[g, :, 0], np.int64) >> 7,
                        minlength=N // 128).max())
        for g in range(nb)
    )
    CB = max(19, -(-maxcnt // 128))
    cfg = _Cfg(N=N, NV=NV, F=F, CB=CB, GB=4)

    in_maps = [_prep_inputs(cfg, x[g], edge_index[g], Ws, bs_) for g in range(nb)]
    nc = _build(cfg)
    res = run_bass_kernel_spmd(nc, in_maps, CORES).results
    out = np.stack([_unpack_output(cfg, res[g]["out_pm"]) for g in range(nb)])
    return out.astype(np.float32)
